# revision 1
# baseline (speedup 1.0000x reference)
"""Banded (sliding-window) multi-head attention for Trainium2, 8 NeuronCores.

Problem: x[4, 2048, 512] -> QKV proj -> RoPE -> banded attention
(window [q-127, q+128]) -> out proj.  See reference.py.

Sharding: (batch n, head-half) -> 8 cores.  Each core computes 4 heads of one
batch end-to-end and a partial out-projection (contraction over its 256 hidden
dims); host gather sums the two partials per batch and adds the bias.

On-core pipeline (all matmuls bf16 with fp32 PSUM accumulation):
  qkT  = Wqk^T.T @ xT        (feat-major, 2 head-packs of 128 partitions)
  RoPE via signed-permutation matmul (rotT = R.T.T @ qkT) + DVE mul/add
  v    = xT.T @ WvT          (token-major, padded key-chunk layout, ones col)
  scoresT[k, q] = kT.T @ qT  per 128-query tile, 3 key chunks (window 384)
  expT = exp(scoresT) ; masked multiplicatively (band+validity, bf16 0/1)
  attn[q, d] (+ sum col via ones column in v) = expT.T @ v
  normalize: recip(sums) per-partition -> tensor_scalar_mul
  PE-transpose attn[q, (2 heads x 64d)] -> attnT[d-pack, q]
  out partial = attnT.T @ owT  -> DMA out fp32
"""

import numpy as np
import ml_dtypes

import concourse.bass as bass
import concourse.bacc as bacc
import concourse.mybir as mybir
import concourse.tile as tile
from concourse import bass_utils

# ---------------- problem constants (hardcoded per contract) ----------------
N_BATCH = 4
T = 2048
D_MODEL = 512
NHEAD = 8
HEAD_DIM = 64           # also rotary dim
WIN_LO, WIN_HI = 127, 128
N_CORES = 8

NT = T // 128           # 16 query tiles of 128
PC = NT + 2             # 18 padded key chunks of 128 (one pad chunk each side)
BF = mybir.dt.bfloat16
F32 = mybir.dt.float32

_CACHE = {}


# ---------------- host-side constant prep ----------------
def _bf16(a):
    return np.ascontiguousarray(a, dtype=np.float32).astype(ml_dtypes.bfloat16)


def _rope_tables():
    # row p of a 128-partition head-pack corresponds to head dim d = p % 64
    d_idx = np.arange(128) % HEAD_DIM
    f_idx = d_idx % (HEAD_DIM // 2)
    invf = 1.0 / (10000.0 ** (np.arange(0, HEAD_DIM, 2, dtype=np.float32) / HEAD_DIM))
    ang = np.arange(T, dtype=np.float32)[None, :] * invf[f_idx][:, None]  # [128, T]
    return _bf16(np.cos(ang)), _bf16(np.sin(ang))


def _rot_matrix_T():
    # rot_qT = R @ qT with R the rotate_half signed permutation (per 64-dim head)
    R = np.zeros((128, 128), np.float32)
    for p in range(128):
        if p % 64 < 32:
            R[p, p + 32] = -1.0
        else:
            R[p, p - 32] = 1.0
    return _bf16(R.T)


def _masks():
    # expT layout per q-tile: [k_part 128, chunk 3, q 128]; q-tile t covers
    # queries t*128+qp, padded key chunk t+c covers keys (t+c-1)*128+kp.
    # band: -127 <= k-q <= 128 with k-q = (c-1)*128 + kp - qp (t-independent).
    kp = np.arange(128)[:, None, None]
    c = np.arange(3)[None, :, None]
    qp = np.arange(128)[None, None, :]
    diff = (c - 1) * 128 + kp - qp
    band = ((diff >= -WIN_LO) & (diff <= WIN_HI)).astype(np.float32)
    first = band.copy()
    first[:, 0, :] = 0.0      # keys < 0
    last = band.copy()
    last[:, 2, :] = 0.0       # keys >= T
    return _bf16(np.stack([first, band, last]))  # [3, 128, 3, 128]


def _prep_weights(Wqkv_w, out_w, half):
    hs = [half * 4 + i for i in range(4)]
    Wq = Wqkv_w[0 * D_MODEL:1 * D_MODEL].reshape(NHEAD, HEAD_DIM, D_MODEL)[hs]
    Wk = Wqkv_w[1 * D_MODEL:2 * D_MODEL].reshape(NHEAD, HEAD_DIM, D_MODEL)[hs]
    Wv = Wqkv_w[2 * D_MODEL:3 * D_MODEL].reshape(NHEAD, HEAD_DIM, D_MODEL)[hs]
    Wq = Wq * np.float32(1.0 / np.sqrt(HEAD_DIM))     # fold attention scale
    # feat order: q(h0,h1), q(h2,h3), k(h0,h1), k(h2,h3)
    wqk = np.concatenate([Wq.reshape(2, 128, D_MODEL), Wk.reshape(2, 128, D_MODEL)], 0)
    wqkT = wqk.reshape(512, D_MODEL).T.reshape(4, 128, 512)       # [xfeat c, 128, feat]
    wvT = Wv.reshape(256, D_MODEL).T.reshape(4, 128, 256)
    owT = out_w[:, half * 256:(half + 1) * 256].T.reshape(2, 128, 512)
    return _bf16(wqkT), _bf16(wvT), _bf16(owT)


# ---------------- bass program ----------------
def build_nc(reps=1):
    import os
    SKIP_ATTN = os.environ.get("KBUILD_SKIP_ATTN") == "1"
    SKIP_PROD = os.environ.get("KBUILD_SKIP_PROD") == "1"
    NO_POOL = os.environ.get("KBUILD_NO_POOL") == "1"
    """reps>1 repeats the whole kernel body (timing harness only)."""
    nc = bacc.Bacc("TRN2", debug=False, enable_asserts=False,
                   target_bir_lowering=False)

    xT_d = nc.dram_tensor("xT", [4, 128, T], BF, kind="ExternalInput")
    wqk_d = nc.dram_tensor("wqk", [4, 128, 512], BF, kind="ExternalInput")
    wv_d = nc.dram_tensor("wv", [4, 128, 256], BF, kind="ExternalInput")
    ow_d = nc.dram_tensor("ow", [2, 128, 512], BF, kind="ExternalInput")
    cos_d = nc.dram_tensor("cosT", [128, T], BF, kind="ExternalInput")
    sin_d = nc.dram_tensor("sinT", [128, T], BF, kind="ExternalInput")
    rt_d = nc.dram_tensor("rotT", [128, 128], BF, kind="ExternalInput")
    id_d = nc.dram_tensor("ident", [128, 128], BF, kind="ExternalInput")
    out_d = nc.dram_tensor("out", [T, 512], F32, kind="ExternalOutput")

    pool_eng = lambda: nc.vector if NO_POOL else nc.gpsimd

    with tile.TileContext(nc) as tc:
        with (
            tc.tile_pool(name="persist", bufs=1) as pers,
            tc.tile_pool(name="work", bufs=3) as work,
            tc.tile_pool(name="expp", bufs=4) as expp,
            tc.tile_pool(name="psbig", bufs=2, space="PSUM") as psbig,
            tc.tile_pool(name="pss", bufs=3, space="PSUM") as pss,
            tc.tile_pool(name="psa", bufs=2, space="PSUM") as psa,
            tc.tile_pool(name="pst", bufs=1, space="PSUM") as pst,
        ):
            # ------- persistent SBUF tensors -------
            xT = pers.tile([128, 4, T], BF)
            wqk = pers.tile([128, 4, 512], BF)
            wv = pers.tile([128, 4, 256], BF)
            ow = pers.tile([128, 2, 512], BF)
            cosb = pers.tile([128, T], BF)
            sinb = pers.tile([128, T], BF)
            rt = pers.tile([128, 128], BF)
            ident = pers.tile([128, 128], BF)
            v4 = pers.tile([128, 4, PC, 65], BF)
            # per-chunk q/k tiles for precise cross-phase dependencies
            qTn = [pers.tile([128, 2, 512], BF, tag=f"qT{n}", name=f"qT{n}")
                   for n in range(4)]
            kTn = [pers.tile([128, 2, 512], BF, tag=f"kT{n}", name=f"kT{n}")
                   for n in range(4)]
            kpad = pers.tile([128, 2, 2, 128], BF)   # [., ., lo/hi, .]

            rep_ctx = tc.For_i(0, reps, 1) if reps > 1 else None
            if rep_ctx is not None:
                rep_ctx.__enter__()
            for rep in range(1):
                # DMA order = first-use order: v/qk production of block 0
                # first, rope tables, remaining token blocks, attention consts
                for c in range(4):
                    nc.sync.dma_start(out=wv[:, c, :], in_=wv_d[c])
                    nc.sync.dma_start(out=wqk[:, c, :], in_=wqk_d[c])
                for c in range(4):
                    nc.sync.dma_start(out=xT[:, c, 0:512], in_=xT_d[c][:, 0:512])
                nc.sync.dma_start(out=rt[:], in_=rt_d[:])
                nc.sync.dma_start(out=cosb[:, 0:512], in_=cos_d[:, 0:512])
                nc.sync.dma_start(out=sinb[:, 0:512], in_=sin_d[:, 0:512])
                for nn in range(1, 4):
                    for c in range(4):
                        nc.sync.dma_start(
                            out=xT[:, c, nn * 512:(nn + 1) * 512],
                            in_=xT_d[c][:, nn * 512:(nn + 1) * 512])
                nc.sync.dma_start(out=cosb[:, 512:], in_=cos_d[:, 512:])
                nc.sync.dma_start(out=sinb[:, 512:], in_=sin_d[:, 512:])
                for c in range(2):
                    nc.sync.dma_start(out=ow[:, c, :], in_=ow_d[c])
                nc.sync.dma_start(out=ident[:], in_=id_d[:])

                # zero padded kv edges (garbage there would poison exp/matmul)
                nc.vector.memset(kpad[:], 0.0)
                nc.vector.memset(v4[:, :, 0, :], 0.0)
                nc.vector.memset(v4[:, :, PC - 1, :], 0.0)
                nc.vector.memset(v4[:, :, 1:PC - 1, 64:65], 1.0)  # ones col -> sums

                def k_ap(pc, rsl, hp):
                    # kT access for padded chunk pc as [rsl, 128] lhsT
                    if pc == 0:
                        return kpad[rsl, hp, 0, :]
                    if pc == PC - 1:
                        return kpad[rsl, hp, 1, :]
                    rc = pc - 1
                    return kTn[rc // 4][rsl, hp, (rc % 4) * 128:(rc % 4 + 1) * 128]

                # ------- fused production + attention, one 512-token block at a time
                def attention_tile(t):
                    if SKIP_ATTN:
                        return
                    # chunks: 0 = keys one block left (mask kp>=qp+1),
                    # 1 = diagonal (fully in-band, NO mask), 2 = right
                    # (mask kp<=qp).  Edge tiles skip their invalid chunk.
                    cs = [c for c in range(3)
                          if not (t == 0 and c == 0) and not (t == NT - 1 and c == 2)]
                    c0, c1 = cs[0], cs[-1]
                    aq = work.tile([128, 2, 2, 64], BF, tag="aq")
                    ps_t = pst.tile([128, 2, 128], BF, tag="small")
                    qt_rhs = qTn[t // 4]
                    qsl = slice((t % 4) * 128, (t % 4 + 1) * 128)
                    for hp in range(2):
                        ps_ss = [pss.tile([128, 3, 128], F32, tag="s", name=f"ps_s{a}")
                                 for a in range(2)]
                        for c in cs:
                            for a in range(2):
                                rsl = slice(a * 64, (a + 1) * 64)
                                nc.tensor.matmul(
                                    ps_ss[a][:, c, :],
                                    lhsT=k_ap(t + c, rsl, hp),
                                    rhs=qt_rhs[rsl, hp, qsl],
                                    start=True, stop=True,
                                )
                        for a in range(2):
                            rsl = slice(a * 64, (a + 1) * 64)
                            ps_s = ps_ss[a]
                            expT = expp.tile([128, 3, 128], BF, tag="expT")
                            nc.scalar.activation(
                                expT[:, c0:c1 + 1, :], ps_s[:, c0:c1 + 1, :],
                                mybir.ActivationFunctionType.Exp)
                            if 0 in cs:
                                pool_eng().affine_select(
                                    out=expT[:, 0, :], in_=expT[:, 0, :],
                                    compare_op=mybir.AluOpType.is_ge,
                                    fill=0.0, base=-1,
                                    pattern=[[-1, 128]], channel_multiplier=1,
                                )
                            if 2 in cs:
                                pool_eng().affine_select(
                                    out=expT[:, 2, :], in_=expT[:, 2, :],
                                    compare_op=mybir.AluOpType.is_ge,
                                    fill=0.0, base=0,
                                    pattern=[[1, 128]], channel_multiplier=-1,
                                )
                            # attn[q, d] + sums col via ones column of v
                            ps_a = psa.tile([128, 2, 65], F32, tag="small2")
                            for i, c in enumerate(cs):
                                nc.tensor.matmul(
                                    ps_a[:, a, :],
                                    lhsT=expT[:, c, :],
                                    rhs=v4[:, hp * 2 + a, t + c, :],
                                    start=(i == 0), stop=(i == len(cs) - 1),
                                )
                            rcp = work.tile([128, 1], F32, tag="rcp")
                            nc.vector.reciprocal_approx_fast(rcp[:], ps_a[:, a, 64:65])
                            nc.vector.tensor_scalar_mul(
                                aq[:, hp, a, :], ps_a[:, a, 0:64], rcp[:])
                        # transpose attn[q, (a d)] -> attnT[(a d), q] per pack
                        nc.tensor.transpose(ps_t[:, hp, :], aq[:, hp, :, :], ident[:])
                    att = work.tile([128, 2, 128], BF, tag="att")
                    nc.scalar.copy(att[:], ps_t[:])
                    # out projection (partial over this core's 256 hidden dims)
                    ps_o = psbig.tile([128, 512], F32, tag="big")
                    for hp in range(2):
                        nc.tensor.matmul(
                            ps_o[:],
                            lhsT=att[:, hp, :],
                            rhs=ow[:, hp, :],
                            start=(hp == 0), stop=(hp == 1),
                        )
                    osb = work.tile([128, 512], F32, tag="osb")
                    nc.vector.tensor_copy(osb[:], ps_o[:])
                    nc.sync.dma_start(out=out_d[t * 128:(t + 1) * 128, :], in_=osb[:])

                def v_tile(t):
                    ps_v = psbig.tile([128, 512], F32, tag="big", name="ps_v")
                    for c in range(4):
                        nc.tensor.matmul(
                            ps_v[:, 0:256],
                            lhsT=xT[:, c, t * 128:(t + 1) * 128],
                            rhs=wv[:, c, :],
                            start=(c == 0), stop=(c == 3),
                        )
                    nc.scalar.copy(
                        v4[:, :, t + 1, 0:64],
                        ps_v[:, 0:256].rearrange("p (h d) -> p h d", h=4),
                    )

                ready = 0
                for n in range(4):
                    nsl = slice(n * 512, (n + 1) * 512)
                    for t in range(4 * n, 4 * n + 4):
                        v_tile(t)
                    for m in range(4 if not SKIP_PROD else 0):
                        ps_qk = psbig.tile([128, 512], F32, tag="big")
                        for c in range(4):
                            nc.tensor.matmul(
                                ps_qk[:],
                                lhsT=wqk[:, c, m * 128:(m + 1) * 128],
                                rhs=xT[:, c, nsl],
                                start=(c == 0), stop=(c == 3),
                            )
                        raw = work.tile([128, 512], BF, tag="raw")
                        nc.vector.tensor_copy(raw[:], ps_qk[:])
                        ps_rot = psbig.tile([128, 512], F32, tag="big")
                        nc.tensor.matmul(ps_rot[:], lhsT=rt[:], rhs=raw[:],
                                         start=True, stop=True)
                        t1 = work.tile([128, 512], BF, tag="t1")
                        pool_eng().tensor_mul(t1[:], raw[:], cosb[:, nsl])
                        t2 = work.tile([128, 512], BF, tag="t2")
                        nc.vector.tensor_mul(t2[:], ps_rot[:], sinb[:, nsl])
                        if m < 2:
                            dest = qTn[n][:, m, :]
                        else:
                            dest = kTn[n][:, m - 2, :]
                        pool_eng().tensor_add(dest, t1[:], t2[:])
                    # attention tiles from the PREVIOUS block's ready set, so their
                    # inputs are complete and engines never stall on them
                    hi = 4 * (n - 1) + 2 if n > 0 else -1
                    while ready <= hi:
                        attention_tile(ready)
                        ready += 1
                while ready < NT:
                    attention_tile(ready)
                    ready += 1

            if rep_ctx is not None:
                rep_ctx.__exit__(None, None, None)

    nc.compile()
    return nc


# ---------------- host prep + run + gather ----------------
def _get_state():
    if "nc" not in _CACHE:
        _CACHE["nc"] = build_nc()
    return _finish_state()


def _finish_state():
    if "cos" not in _CACHE:
        _CACHE["cos"], _CACHE["sin"] = _rope_tables()
        _CACHE["rotT"] = _rot_matrix_T()
        _CACHE["ident"] = _bf16(np.eye(128, dtype=np.float32))
    return _CACHE


def make_in_maps(x, Wqkv_w, out_w):
    st = _get_state()
    halves = [_prep_weights(Wqkv_w, out_w, h) for h in range(2)]
    in_maps = []
    for core in range(N_CORES):
        n, half = core // 2, core % 2
        wqkT, wvT, owT = halves[half]
        xT = _bf16(x[n].T).reshape(4, 128, T)
        in_maps.append({
            "xT": xT, "wqk": wqkT, "wv": wvT, "ow": owT,
            "cosT": st["cos"], "sinT": st["sin"],
            "rotT": st["rotT"], "ident": st["ident"],
        })
    return in_maps


def gather(results, out_b, dtype):
    outs = []
    for n in range(N_BATCH):
        o = results[2 * n]["out"] + results[2 * n + 1]["out"] + out_b[None, :]
        outs.append(o)
    return np.stack(outs).astype(dtype, copy=False)


def kernel(x, Wqkv_w, out_w, out_b):
    x = np.asarray(x)
    st = _get_state()
    in_maps = make_in_maps(x, np.asarray(Wqkv_w), np.asarray(out_w))
    res = bass_utils.run_bass_kernel_spmd(
        st["nc"], in_maps, core_ids=list(range(N_CORES)))
    return gather(res.results, np.asarray(out_b), x.dtype)



# revision 5
# speedup vs baseline: 1.1842x; 1.1842x over previous
"""Banded (sliding-window) multi-head attention for Trainium2, 8 NeuronCores.

Problem: x[4, 2048, 512] -> QKV proj -> RoPE -> banded attention
(window [q-127, q+128]) -> out proj.  See reference.py.

Sharding: (batch n, head-half) -> 8 cores.  Each core computes 4 heads of one
batch end-to-end and a partial out-projection (contraction over its 256 hidden
dims); host gather sums the two partials per batch and adds the bias.

On-core pipeline (matmuls bf16, fp32 PSUM accumulation for qk):
  qkT  = Wqk^T.T @ xT        (feat-major, 2 head-packs of 128 partitions)
  RoPE via signed-permutation matmul (rotT = R.T.T @ qkT) + DVE/gpsimd mul/add
  v    = xT.T @ WvT          (token-major, 16 key chunks, ones col for sums)
  scores, key-chunk-stationary: per key chunk rc, scoresT[k, 3 q-tiles]
         = kT_rc.T @ qT[rc-1..rc+1]  (two row-tiled head matmuls, N<=384)
  expT = exp(scoresT) both heads in one activation; band masks applied
         multiplicatively on DVE (bf16 constant tiles)
  attn[q, d] (+ sums via ones column in v) = expT.T @ v   per q tile
  normalize: recip(sums) -> per-partition scale (DVE a=0, ACT a=1)
  PE-transpose attn[q, (2 heads x 64d)] -> attnT[d-pack, q]
  out partial = attnT.T @ owT -> bf16 psum -> DMA out bf16, host upcasts
"""

import numpy as np
import ml_dtypes

import concourse.bass as bass
import concourse.bacc as bacc
import concourse.mybir as mybir
import concourse.tile as tile
from concourse import bass_utils

# ---------------- problem constants (hardcoded per contract) ----------------
N_BATCH = 4
T = 2048
D_MODEL = 512
NHEAD = 8
HEAD_DIM = 64           # also rotary dim
WIN_LO, WIN_HI = 127, 128
N_CORES = 8

NT = T // 128           # 16 query tiles / key chunks of 128
BF = mybir.dt.bfloat16
F32 = mybir.dt.float32

_CACHE = {}


# ---------------- host-side constant prep ----------------
def _bf16(a):
    return np.ascontiguousarray(a, dtype=np.float32).astype(ml_dtypes.bfloat16)


def _rope_tables():
    # row p of a 128-partition head-pack corresponds to head dim d = p % 64
    d_idx = np.arange(128) % HEAD_DIM
    f_idx = d_idx % (HEAD_DIM // 2)
    invf = 1.0 / (10000.0 ** (np.arange(0, HEAD_DIM, 2, dtype=np.float32) / HEAD_DIM))
    ang = np.arange(T, dtype=np.float32)[None, :] * invf[f_idx][:, None]  # [128, T]
    return _bf16(np.cos(ang)), _bf16(np.sin(ang))


def _rot_matrix_T():
    # rot_qT = R @ qT with R the rotate_half signed permutation (per 64-dim head)
    R = np.zeros((128, 128), np.float32)
    for p in range(128):
        if p % 64 < 32:
            R[p, p + 32] = -1.0
        else:
            R[p, p - 32] = 1.0
    return _bf16(R.T)


def _mask_blocks():
    # expT_rc block b holds scoresT[key chunk rc, q tile rc-1+b]; k - q offset
    # is 128*(1-b).  Block 0 (keys one chunk right of queries): keep kp <= qp.
    # Block 2 (keys one chunk left): keep kp >= qp+1.  Block 1 (diag): in-band.
    kp = np.arange(128)[:, None]
    qp = np.arange(128)[None, :]
    m0 = (kp <= qp).astype(np.float32)        # right chunk
    m1 = np.ones((128, 128), np.float32)      # diagonal, fully in band
    m2 = (kp >= qp + 1).astype(np.float32)    # left chunk
    m = np.stack([m0, m1, m2])                # [3, 128, 128]
    # duplicate for both heads of a pack: [128, 2(a), 3(b), 128]
    return _bf16(np.broadcast_to(m[None], (2, 3, 128, 128)).transpose(2, 0, 1, 3))


def _prep_weights(Wqkv_w, out_w, half):
    hs = [half * 4 + i for i in range(4)]
    Wq = Wqkv_w[0 * D_MODEL:1 * D_MODEL].reshape(NHEAD, HEAD_DIM, D_MODEL)[hs]
    Wk = Wqkv_w[1 * D_MODEL:2 * D_MODEL].reshape(NHEAD, HEAD_DIM, D_MODEL)[hs]
    Wv = Wqkv_w[2 * D_MODEL:3 * D_MODEL].reshape(NHEAD, HEAD_DIM, D_MODEL)[hs]
    Wq = Wq * np.float32(1.0 / np.sqrt(HEAD_DIM))     # fold attention scale
    # feat order: q(h0,h1), q(h2,h3), k(h0,h1), k(h2,h3)
    wqk = np.concatenate([Wq.reshape(2, 128, D_MODEL), Wk.reshape(2, 128, D_MODEL)], 0)
    wqkT = wqk.reshape(512, D_MODEL).T.reshape(4, 128, 512)       # [xfeat c, 128, feat]
    wvT = Wv.reshape(256, D_MODEL).T.reshape(4, 128, 256)
    owT = out_w[:, half * 256:(half + 1) * 256].T.reshape(2, 128, 512)
    return _bf16(wqkT), _bf16(wvT), _bf16(owT)


# ---------------- bass program ----------------
def build_nc():
    nc = bacc.Bacc("TRN2", debug=False, enable_asserts=False,
                   target_bir_lowering=False)

    xT_d = nc.dram_tensor("xT", [4, 128, T], BF, kind="ExternalInput")
    wqk_d = nc.dram_tensor("wqk", [4, 128, 512], BF, kind="ExternalInput")
    wv_d = nc.dram_tensor("wv", [4, 128, 256], BF, kind="ExternalInput")
    ow_d = nc.dram_tensor("ow", [2, 128, 512], BF, kind="ExternalInput")
    cos_d = nc.dram_tensor("cosT", [128, T], BF, kind="ExternalInput")
    sin_d = nc.dram_tensor("sinT", [128, T], BF, kind="ExternalInput")
    rt_d = nc.dram_tensor("rotT", [128, 128], BF, kind="ExternalInput")
    id_d = nc.dram_tensor("ident", [128, 128], BF, kind="ExternalInput")
    msk_d = nc.dram_tensor("maskblk", [128, 2, 3, 128], BF, kind="ExternalInput")
    out_d = nc.dram_tensor("out", [T, 512], BF, kind="ExternalOutput")

    with tile.TileContext(nc) as tc:
        with (
            tc.tile_pool(name="persist", bufs=1) as pers,
            tc.tile_pool(name="work", bufs=3) as work,
            tc.tile_pool(name="expp", bufs=8) as expp,
            tc.tile_pool(name="psbig", bufs=2, space="PSUM") as psbig,
            tc.tile_pool(name="pssc", bufs=1, space="PSUM") as pssc,
            tc.tile_pool(name="psa", bufs=2, space="PSUM") as psa,
            tc.tile_pool(name="pst", bufs=1, space="PSUM") as pst,
            tc.tile_pool(name="pso", bufs=1, space="PSUM") as pso,
        ):
            # ------- persistent SBUF tensors -------
            xT = pers.tile([128, 4, T], BF)
            wqk = pers.tile([128, 4, 512], BF)
            wv = pers.tile([128, 4, 256], BF)
            ow = pers.tile([128, 2, 512], BF)
            cosb = pers.tile([128, T], BF)
            sinb = pers.tile([128, T], BF)
            rt = pers.tile([128, 128], BF)
            ident = pers.tile([128, 128], BF)
            mskb = pers.tile([128, 2, 3, 128], BF)
            v4 = pers.tile([128, 4, NT, 65], BF)
            qT = pers.tile([128, 2, T], BF)
            kT = pers.tile([128, 2, T], BF)

            # DMA order = first-use order
            for c in range(4):
                nc.sync.dma_start(out=wv[:, c, :], in_=wv_d[c])
                nc.sync.dma_start(out=wqk[:, c, :], in_=wqk_d[c])
            for c in range(4):
                nc.sync.dma_start(out=xT[:, c, 0:512], in_=xT_d[c][:, 0:512])
            nc.sync.dma_start(out=rt[:], in_=rt_d[:])
            nc.sync.dma_start(out=cosb[:, 0:512], in_=cos_d[:, 0:512])
            nc.sync.dma_start(out=sinb[:, 0:512], in_=sin_d[:, 0:512])
            for nn in range(1, 4):
                for c in range(4):
                    nc.sync.dma_start(
                        out=xT[:, c, nn * 512:(nn + 1) * 512],
                        in_=xT_d[c][:, nn * 512:(nn + 1) * 512])
            nc.sync.dma_start(out=cosb[:, 512:], in_=cos_d[:, 512:])
            nc.sync.dma_start(out=sinb[:, 512:], in_=sin_d[:, 512:])
            nc.sync.dma_start(out=mskb[:], in_=msk_d[:])
            for c in range(2):
                nc.sync.dma_start(out=ow[:, c, :], in_=ow_d[c])
            nc.sync.dma_start(out=ident[:], in_=id_d[:])

            nc.vector.memset(v4[:, :, :, 64:65], 1.0)   # ones col -> sums

            # ------- production: qkv proj + rope for one 512-token block -----
            def v_tile(t):
                ps_v = psbig.tile([128, 256], F32, tag="big", name="ps_v")
                for c in range(4):
                    nc.tensor.matmul(
                        ps_v[:],
                        lhsT=xT[:, c, t * 128:(t + 1) * 128],
                        rhs=wv[:, c, :],
                        start=(c == 0), stop=(c == 3),
                    )
                nc.vector.tensor_copy(
                    v4[:, :, t, 0:64],
                    ps_v[:].rearrange("p (h d) -> p h d", h=4),
                )

            def qk_block(n):
                nsl = slice(n * 512, (n + 1) * 512)
                for m in range(4):
                    ps_qk = psbig.tile([128, 512], F32, tag="big")
                    for c in range(4):
                        nc.tensor.matmul(
                            ps_qk[:],
                            lhsT=wqk[:, c, m * 128:(m + 1) * 128],
                            rhs=xT[:, c, nsl],
                            start=(c == 0), stop=(c == 3),
                        )
                    raw = work.tile([128, 512], BF, tag="raw")
                    nc.scalar.copy(raw[:], ps_qk[:])
                    ps_rot = psbig.tile([128, 512], F32, tag="big")
                    nc.tensor.matmul(ps_rot[:], lhsT=rt[:], rhs=raw[:],
                                     start=True, stop=True)
                    t1 = work.tile([128, 512], BF, tag="t1")
                    nc.gpsimd.tensor_mul(t1[:], raw[:], cosb[:, nsl])
                    t2 = work.tile([128, 512], BF, tag="t2")
                    nc.vector.tensor_mul(t2[:], ps_rot[:], sinb[:, nsl])
                    dest = qT[:, m, nsl] if m < 2 else kT[:, m - 2, nsl]
                    nc.gpsimd.tensor_add(dest, t1[:], t2[:])

            # ------- attention: key-chunk-stationary scores + exp + mask -----
            expT = {}

            def score_chunk(rc):
                lo_t = max(rc - 1, 0)
                hi_t = min(rc + 1, NT - 1)
                b0 = lo_t - (rc - 1)           # 0 or 1
                nb = hi_t - lo_t + 1           # 2 or 3
                csl = slice(b0 * 128, (b0 + nb) * 128)
                ksl = slice(rc * 128, (rc + 1) * 128)
                qsl = slice(lo_t * 128, (hi_t + 1) * 128)
                for hp in range(2):
                    ps_s = pssc.tile([128, 2, 512], F32, tag="sc")
                    for a in range(2):
                        rsl = slice(a * 64, (a + 1) * 64)
                        nc.tensor.matmul(
                            ps_s[:, a, csl],
                            lhsT=kT[rsl, hp, ksl],
                            rhs=qT[rsl, hp, qsl],
                            start=True, stop=True,
                        )
                    ex = expp.tile([128, 2, 384], BF, tag="expT",
                                   name=f"ex{rc}_{hp}")
                    nc.scalar.activation(
                        ex[:, :, csl], ps_s[:, :, csl],
                        mybir.ActivationFunctionType.Exp)
                    # band masks, multiplicative (both heads in one op)
                    exv = ex[:].rearrange("p a (b q) -> p a b q", q=128)
                    if rc > 0 and rc < NT - 1:
                        nc.vector.tensor_mul(exv, exv, mskb[:])
                    elif rc == 0:
                        nc.vector.tensor_mul(
                            exv[:, :, 1:3, :], exv[:, :, 1:3, :],
                            mskb[:, :, 1:3, :])
                    else:
                        nc.vector.tensor_mul(
                            exv[:, :, 0:2, :], exv[:, :, 0:2, :],
                            mskb[:, :, 0:2, :])
                    expT[(rc, hp)] = ex

            # ------- attention: attn@v, normalize, transpose, out proj ------
            def attn_tile(t):
                cs = [c for c in range(3)
                      if not (t == 0 and c == 0) and not (t == NT - 1 and c == 2)]
                aq = work.tile([128, 2, 2, 64], BF, tag="aq")
                ps_t = pst.tile([128, 2, 128], BF, tag="small")
                for hp in range(2):
                    ps_a = psa.tile([128, 2, 65], F32, tag="small2")
                    for a in range(2):
                        for i, c in enumerate(cs):
                            ex = expT[(t - 1 + c, hp)]
                            nc.tensor.matmul(
                                ps_a[:, a, :],
                                lhsT=ex[:, a, (2 - c) * 128:(3 - c) * 128],
                                rhs=v4[:, hp * 2 + a, t - 1 + c, :],
                                start=(i == 0), stop=(i == len(cs) - 1),
                            )
                    rcp = work.tile([128, 2, 1], F32, tag="rcp")
                    nc.vector.reciprocal_approx_fast(rcp[:], ps_a[:, :, 64:65])
                    nc.vector.tensor_scalar_mul(
                        aq[:, hp, 0, :], ps_a[:, 0, 0:64], rcp[:, 0, :])
                    nc.scalar.mul(aq[:, hp, 1, :], ps_a[:, 1, 0:64], rcp[:, 1, :])
                    nc.tensor.transpose(ps_t[:, hp, :], aq[:, hp, :, :], ident[:])
                att = work.tile([128, 2, 128], BF, tag="att")
                nc.vector.tensor_copy(att[:], ps_t[:])
                ps_o = pso.tile([128, 512], F32, tag="out")
                for hp in range(2):
                    nc.tensor.matmul(
                        ps_o[:],
                        lhsT=att[:, hp, :],
                        rhs=ow[:, hp, :],
                        start=(hp == 0), stop=(hp == 1),
                    )
                osb = work.tile([128, 512], BF, tag="osb")
                if t % 2 == 0:
                    nc.vector.tensor_copy(osb[:], ps_o[:])
                else:
                    nc.scalar.copy(osb[:], ps_o[:])
                nc.sync.dma_start(out=out_d[t * 128:(t + 1) * 128, :], in_=osb[:])

            # ------- schedule: production block n, then trailing attention ---
            sc_done = -1
            at_done = -1
            for n in range(4):
                for t in range(4 * n, 4 * n + 4):
                    v_tile(t)
                qk_block(n)
                hi_rc = 4 * n + 2 if n < 3 else NT - 1
                while sc_done < hi_rc:
                    sc_done += 1
                    score_chunk(sc_done)
                    while at_done < sc_done - 1:
                        at_done += 1
                        attn_tile(at_done)
            while at_done < NT - 1:
                at_done += 1
                attn_tile(at_done)

    nc.compile()
    return nc


# ---------------- host prep + run + gather ----------------
def _get_state():
    if "nc" not in _CACHE:
        _CACHE["nc"] = build_nc()
    if "cos" not in _CACHE:
        _CACHE["cos"], _CACHE["sin"] = _rope_tables()
        _CACHE["rotT"] = _rot_matrix_T()
        _CACHE["ident"] = _bf16(np.eye(128, dtype=np.float32))
        _CACHE["maskblk"] = _mask_blocks()
    return _CACHE


def make_in_maps(x, Wqkv_w, out_w):
    st = _get_state()
    halves = [_prep_weights(Wqkv_w, out_w, h) for h in range(2)]
    in_maps = []
    for core in range(N_CORES):
        n, half = core // 2, core % 2
        wqkT, wvT, owT = halves[half]
        xT = _bf16(x[n].T).reshape(4, 128, T)
        in_maps.append({
            "xT": xT, "wqk": wqkT, "wv": wvT, "ow": owT,
            "cosT": st["cos"], "sinT": st["sin"],
            "rotT": st["rotT"], "ident": st["ident"],
            "maskblk": st["maskblk"],
        })
    return in_maps


def gather(results, out_b, dtype):
    outs = []
    for n in range(N_BATCH):
        o = (results[2 * n]["out"].astype(np.float32)
             + results[2 * n + 1]["out"].astype(np.float32) + out_b[None, :])
        outs.append(o)
    return np.stack(outs).astype(dtype, copy=False)


def kernel(x, Wqkv_w, out_w, out_b):
    x = np.asarray(x)
    st = _get_state()
    in_maps = make_in_maps(x, np.asarray(Wqkv_w), np.asarray(out_w))
    res = bass_utils.run_bass_kernel_spmd(
        st["nc"], in_maps, core_ids=list(range(N_CORES)))
    return gather(res.results, np.asarray(out_b), x.dtype)


# revision 6
# speedup vs baseline: 1.2573x; 1.0618x over previous
"""Banded (sliding-window) multi-head attention for Trainium2, 8 NeuronCores.

Problem: x[4, 2048, 512] -> QKV proj -> RoPE -> banded attention
(window [q-127, q+128]) -> out proj.  See reference.py.

Sharding: (batch n, head-half) -> 8 cores.  Each core computes 4 heads of one
batch end-to-end and a partial out-projection (contraction over its 256 hidden
dims); host gather sums the two partials per batch and adds the bias.

On-core pipeline (matmuls bf16, fp32 PSUM accumulation for qk):
  qkT  = Wqk^T.T @ xT        (feat-major, 2 head-packs of 128 partitions)
  RoPE via signed-permutation matmul (rotT = R.T.T @ qkT) + DVE/gpsimd mul/add
  v    = xT.T @ WvT          (token-major, 16 key chunks, ones col for sums)
  scores, key-chunk-stationary: per key chunk rc, scoresT[k, 3 q-tiles]
         = kT_rc.T @ qT[rc-1..rc+1]  (two row-tiled head matmuls, N<=384)
  expT = exp(scoresT) both heads in one activation; band masks applied
         multiplicatively on DVE (bf16 constant tiles)
  attn[q, d] (+ sums via ones column in v) = expT.T @ v   per q tile
  normalize: recip(sums) -> per-partition scale (DVE a=0, ACT a=1)
  PE-transpose attn[q, (2 heads x 64d)] -> attnT[d-pack, q]
  out partial = attnT.T @ owT -> bf16 psum -> DMA out bf16, host upcasts
"""

import numpy as np
import ml_dtypes

import concourse.bass as bass
import concourse.bacc as bacc
import concourse.mybir as mybir
import concourse.tile as tile
from concourse import bass_utils

# ---------------- problem constants (hardcoded per contract) ----------------
N_BATCH = 4
T = 2048
D_MODEL = 512
NHEAD = 8
HEAD_DIM = 64           # also rotary dim
WIN_LO, WIN_HI = 127, 128
N_CORES = 8

NT = T // 128           # 16 query tiles / key chunks of 128
BF = mybir.dt.bfloat16
F32 = mybir.dt.float32

_CACHE = {}


# ---------------- host-side constant prep ----------------
def _bf16(a):
    return np.ascontiguousarray(a, dtype=np.float32).astype(ml_dtypes.bfloat16)


def _rope_tables():
    # row p of a 128-partition head-pack corresponds to head dim d = p % 64
    d_idx = np.arange(128) % HEAD_DIM
    f_idx = d_idx % (HEAD_DIM // 2)
    invf = 1.0 / (10000.0 ** (np.arange(0, HEAD_DIM, 2, dtype=np.float32) / HEAD_DIM))
    ang = np.arange(T, dtype=np.float32)[None, :] * invf[f_idx][:, None]  # [128, T]
    return _bf16(np.cos(ang)), _bf16(np.sin(ang))


def _rot_matrix_T():
    # rot_qT = R @ qT with R the rotate_half signed permutation (per 64-dim head)
    R = np.zeros((128, 128), np.float32)
    for p in range(128):
        if p % 64 < 32:
            R[p, p + 32] = -1.0
        else:
            R[p, p - 32] = 1.0
    return _bf16(R.T)


def _mask_blocks():
    # expT_rc block b holds scoresT[key chunk rc, q tile rc-1+b]; k - q offset
    # is 128*(1-b).  Block 0 (keys one chunk right of queries): keep kp <= qp.
    # Block 2 (keys one chunk left): keep kp >= qp+1.  Block 1 (diag): in-band.
    kp = np.arange(128)[:, None]
    qp = np.arange(128)[None, :]
    m0 = (kp <= qp).astype(np.float32)        # right chunk
    m1 = np.ones((128, 128), np.float32)      # diagonal, fully in band
    m2 = (kp >= qp + 1).astype(np.float32)    # left chunk
    m = np.stack([m0, m1, m2])                # [3, 128, 128]
    # duplicate for both heads of a pack: [128, 2(a), 3(b), 128]
    return _bf16(np.broadcast_to(m[None], (2, 3, 128, 128)).transpose(2, 0, 1, 3))


def _prep_weights(Wqkv_w, out_w, half):
    hs = [half * 4 + i for i in range(4)]
    Wq = Wqkv_w[0 * D_MODEL:1 * D_MODEL].reshape(NHEAD, HEAD_DIM, D_MODEL)[hs]
    Wk = Wqkv_w[1 * D_MODEL:2 * D_MODEL].reshape(NHEAD, HEAD_DIM, D_MODEL)[hs]
    Wv = Wqkv_w[2 * D_MODEL:3 * D_MODEL].reshape(NHEAD, HEAD_DIM, D_MODEL)[hs]
    Wq = Wq * np.float32(1.0 / np.sqrt(HEAD_DIM))     # fold attention scale
    # feat order: q(h0,h1), q(h2,h3), k(h0,h1), k(h2,h3)
    wqk = np.concatenate([Wq.reshape(2, 128, D_MODEL), Wk.reshape(2, 128, D_MODEL)], 0)
    wqkT = wqk.reshape(512, D_MODEL).T.reshape(4, 128, 512)       # [xfeat c, 128, feat]
    wvT = Wv.reshape(256, D_MODEL).T.reshape(4, 128, 256)
    owT = out_w[:, half * 256:(half + 1) * 256].T.reshape(2, 128, 512)
    return _bf16(wqkT), _bf16(wvT), _bf16(owT)


# ---------------- bass program ----------------
def build_nc():
    nc = bacc.Bacc("TRN2", debug=False, enable_asserts=False,
                   target_bir_lowering=False)

    xT_d = nc.dram_tensor("xT", [4, 128, T], BF, kind="ExternalInput")
    wqk_d = nc.dram_tensor("wqk", [4, 128, 512], BF, kind="ExternalInput")
    wv_d = nc.dram_tensor("wv", [4, 128, 256], BF, kind="ExternalInput")
    ow_d = nc.dram_tensor("ow", [2, 128, 512], BF, kind="ExternalInput")
    cos_d = nc.dram_tensor("cosT", [128, T], BF, kind="ExternalInput")
    sin_d = nc.dram_tensor("sinT", [128, T], BF, kind="ExternalInput")
    rt_d = nc.dram_tensor("rotT", [128, 128], BF, kind="ExternalInput")
    id_d = nc.dram_tensor("ident", [128, 128], BF, kind="ExternalInput")
    msk_d = nc.dram_tensor("maskblk", [128, 2, 3, 128], BF, kind="ExternalInput")
    out_d = nc.dram_tensor("out", [T, 512], BF, kind="ExternalOutput")

    with tile.TileContext(nc) as tc:
        with (
            tc.tile_pool(name="persist", bufs=1) as pers,
            tc.tile_pool(name="work", bufs=3) as work,
            tc.tile_pool(name="expp", bufs=8) as expp,
            tc.tile_pool(name="psbig", bufs=2, space="PSUM") as psbig,
            tc.tile_pool(name="pssc", bufs=1, space="PSUM") as pssc,
            tc.tile_pool(name="psa", bufs=2, space="PSUM") as psa,
            tc.tile_pool(name="pst", bufs=1, space="PSUM") as pst,
            tc.tile_pool(name="pso", bufs=1, space="PSUM") as pso,
        ):
            # ------- persistent SBUF tensors -------
            xT = pers.tile([128, 4, T], BF)
            wqk = pers.tile([128, 4, 512], BF)
            wv = pers.tile([128, 4, 256], BF)
            ow = pers.tile([128, 2, 512], BF)
            cosb = pers.tile([128, T], BF)
            sinb = pers.tile([128, T], BF)
            rt = pers.tile([128, 128], BF)
            ident = pers.tile([128, 128], BF)
            mskb = pers.tile([128, 2, 3, 128], BF)
            v4 = pers.tile([128, 4, NT, 65], BF)
            qT = pers.tile([128, 2, T], BF)
            kT = pers.tile([128, 2, T], BF)

            # DMA order = first-use order
            for c in range(4):
                nc.sync.dma_start(out=wv[:, c, :], in_=wv_d[c])
                nc.sync.dma_start(out=xT[:, c, 0:512], in_=xT_d[c][:, 0:512])
            for c in range(4):
                nc.sync.dma_start(out=wqk[:, c, :], in_=wqk_d[c])
            nc.sync.dma_start(out=rt[:], in_=rt_d[:])
            nc.sync.dma_start(out=cosb[:, 0:512], in_=cos_d[:, 0:512])
            nc.sync.dma_start(out=sinb[:, 0:512], in_=sin_d[:, 0:512])
            for nn in range(1, 4):
                for c in range(4):
                    nc.sync.dma_start(
                        out=xT[:, c, nn * 512:(nn + 1) * 512],
                        in_=xT_d[c][:, nn * 512:(nn + 1) * 512])
            nc.sync.dma_start(out=cosb[:, 512:], in_=cos_d[:, 512:])
            nc.sync.dma_start(out=sinb[:, 512:], in_=sin_d[:, 512:])
            nc.sync.dma_start(out=mskb[:], in_=msk_d[:])
            for c in range(2):
                nc.sync.dma_start(out=ow[:, c, :], in_=ow_d[c])
            nc.sync.dma_start(out=ident[:], in_=id_d[:])

            nc.vector.memset(v4[:, :, :, 64:65], 1.0)   # ones col -> sums

            # ------- production: qkv proj + rope for one 512-token block -----
            def v_tile(t):
                ps_v = psbig.tile([128, 256], F32, tag="big", name="ps_v")
                for c in range(4):
                    nc.tensor.matmul(
                        ps_v[:],
                        lhsT=xT[:, c, t * 128:(t + 1) * 128],
                        rhs=wv[:, c, :],
                        start=(c == 0), stop=(c == 3),
                    )
                nc.vector.tensor_copy(
                    v4[:, :, t, 0:64],
                    ps_v[:].rearrange("p (h d) -> p h d", h=4),
                )

            def qk_block(n):
                # rope for pack m trails the qk matmuls of pack m+1, so the
                # rot matmul never stalls PE on the scalar psum->sbuf cast
                nsl = slice(n * 512, (n + 1) * 512)
                raws = {}

                def rope_tail(m):
                    raw = raws.pop(m)
                    ps_rot = psbig.tile([128, 512], F32, tag="big")
                    nc.tensor.matmul(ps_rot[:], lhsT=rt[:], rhs=raw[:],
                                     start=True, stop=True)
                    t1 = work.tile([128, 512], BF, tag="t1")
                    nc.gpsimd.tensor_mul(t1[:], raw[:], cosb[:, nsl])
                    t2 = work.tile([128, 512], BF, tag="t2")
                    nc.vector.tensor_mul(t2[:], ps_rot[:], sinb[:, nsl])
                    dest = qT[:, m, nsl] if m < 2 else kT[:, m - 2, nsl]
                    nc.gpsimd.tensor_add(dest, t1[:], t2[:])

                for m in range(4):
                    ps_qk = psbig.tile([128, 512], F32, tag="big")
                    for c in range(4):
                        nc.tensor.matmul(
                            ps_qk[:],
                            lhsT=wqk[:, c, m * 128:(m + 1) * 128],
                            rhs=xT[:, c, nsl],
                            start=(c == 0), stop=(c == 3),
                        )
                    raw = work.tile([128, 512], BF, tag="raw")
                    nc.scalar.copy(raw[:], ps_qk[:])
                    raws[m] = raw
                    if m > 0:
                        rope_tail(m - 1)
                rope_tail(3)

            # ------- attention: key-chunk-stationary scores + exp + mask -----
            expT = {}

            def score_chunk(rc):
                lo_t = max(rc - 1, 0)
                hi_t = min(rc + 1, NT - 1)
                b0 = lo_t - (rc - 1)           # 0 or 1
                nb = hi_t - lo_t + 1           # 2 or 3
                csl = slice(b0 * 128, (b0 + nb) * 128)
                ksl = slice(rc * 128, (rc + 1) * 128)
                qsl = slice(lo_t * 128, (hi_t + 1) * 128)
                for hp in range(2):
                    ps_s = pssc.tile([128, 2, 512], F32, tag="sc")
                    for a in range(2):
                        rsl = slice(a * 64, (a + 1) * 64)
                        nc.tensor.matmul(
                            ps_s[:, a, csl],
                            lhsT=kT[rsl, hp, ksl],
                            rhs=qT[rsl, hp, qsl],
                            start=True, stop=True,
                        )
                    ex = expp.tile([128, 2, 384], BF, tag="expT",
                                   name=f"ex{rc}_{hp}")
                    nc.scalar.activation(
                        ex[:, :, csl], ps_s[:, :, csl],
                        mybir.ActivationFunctionType.Exp)
                    # band masks, multiplicative (both heads in one op);
                    # block 1 (diagonal) is fully in-band, no mask needed
                    exv = ex[:].rearrange("p a (b q) -> p a b q", q=128)
                    if rc > 0 and rc < NT - 1:
                        nc.vector.tensor_mul(
                            exv[:, :, 0:3:2, :], exv[:, :, 0:3:2, :],
                            mskb[:, :, 0:3:2, :])
                    elif rc == 0:
                        nc.vector.tensor_mul(
                            exv[:, :, 2:3, :], exv[:, :, 2:3, :],
                            mskb[:, :, 2:3, :])
                    else:
                        nc.vector.tensor_mul(
                            exv[:, :, 0:1, :], exv[:, :, 0:1, :],
                            mskb[:, :, 0:1, :])
                    expT[(rc, hp)] = ex

            # ------- attention: attn@v, normalize, transpose, out proj ------
            def attn_tile(t):
                cs = [c for c in range(3)
                      if not (t == 0 and c == 0) and not (t == NT - 1 and c == 2)]
                aq = work.tile([128, 2, 2, 64], BF, tag="aq")
                ps_t = pst.tile([128, 2, 128], BF, tag="small")
                for hp in range(2):
                    ps_a = psa.tile([128, 2, 65], F32, tag="small2")
                    for a in range(2):
                        for i, c in enumerate(cs):
                            ex = expT[(t - 1 + c, hp)]
                            nc.tensor.matmul(
                                ps_a[:, a, :],
                                lhsT=ex[:, a, (2 - c) * 128:(3 - c) * 128],
                                rhs=v4[:, hp * 2 + a, t - 1 + c, :],
                                start=(i == 0), stop=(i == len(cs) - 1),
                            )
                    rcp = work.tile([128, 2, 1], F32, tag="rcp")
                    nc.vector.reciprocal_approx_fast(rcp[:], ps_a[:, :, 64:65])
                    nc.vector.tensor_scalar_mul(
                        aq[:, hp, 0, :], ps_a[:, 0, 0:64], rcp[:, 0, :])
                    nc.scalar.mul(aq[:, hp, 1, :], ps_a[:, 1, 0:64], rcp[:, 1, :])
                    nc.tensor.transpose(ps_t[:, hp, :], aq[:, hp, :, :], ident[:])
                att = work.tile([128, 2, 128], BF, tag="att")
                nc.vector.tensor_copy(att[:], ps_t[:])
                ps_o = pso.tile([128, 512], F32, tag="out")
                for hp in range(2):
                    nc.tensor.matmul(
                        ps_o[:],
                        lhsT=att[:, hp, :],
                        rhs=ow[:, hp, :],
                        start=(hp == 0), stop=(hp == 1),
                    )
                osb = work.tile([128, 512], BF, tag="osb")
                if t % 2 == 0:
                    nc.vector.tensor_copy(osb[:], ps_o[:])
                else:
                    nc.scalar.copy(osb[:], ps_o[:])
                nc.sync.dma_start(out=out_d[t * 128:(t + 1) * 128, :], in_=osb[:])

            # ------- schedule: production block n, then trailing attention ---
            sc_done = -1
            at_done = -1
            for n in range(4):
                for t in range(4 * n, 4 * n + 4):
                    v_tile(t)
                qk_block(n)
                hi_rc = 4 * n + 2 if n < 3 else NT - 1
                while sc_done < hi_rc:
                    sc_done += 1
                    score_chunk(sc_done)
                    while at_done < sc_done - 1:
                        at_done += 1
                        attn_tile(at_done)
            while at_done < NT - 1:
                at_done += 1
                attn_tile(at_done)

    nc.compile()
    return nc


# ---------------- host prep + run + gather ----------------
def _get_state():
    if "nc" not in _CACHE:
        _CACHE["nc"] = build_nc()
    if "cos" not in _CACHE:
        _CACHE["cos"], _CACHE["sin"] = _rope_tables()
        _CACHE["rotT"] = _rot_matrix_T()
        _CACHE["ident"] = _bf16(np.eye(128, dtype=np.float32))
        _CACHE["maskblk"] = _mask_blocks()
    return _CACHE


def make_in_maps(x, Wqkv_w, out_w):
    st = _get_state()
    halves = [_prep_weights(Wqkv_w, out_w, h) for h in range(2)]
    in_maps = []
    for core in range(N_CORES):
        n, half = core // 2, core % 2
        wqkT, wvT, owT = halves[half]
        xT = _bf16(x[n].T).reshape(4, 128, T)
        in_maps.append({
            "xT": xT, "wqk": wqkT, "wv": wvT, "ow": owT,
            "cosT": st["cos"], "sinT": st["sin"],
            "rotT": st["rotT"], "ident": st["ident"],
            "maskblk": st["maskblk"],
        })
    return in_maps


def gather(results, out_b, dtype):
    outs = []
    for n in range(N_BATCH):
        o = (results[2 * n]["out"].astype(np.float32)
             + results[2 * n + 1]["out"].astype(np.float32) + out_b[None, :])
        outs.append(o)
    return np.stack(outs).astype(dtype, copy=False)


def kernel(x, Wqkv_w, out_w, out_b):
    x = np.asarray(x)
    st = _get_state()
    in_maps = make_in_maps(x, np.asarray(Wqkv_w), np.asarray(out_w))
    res = bass_utils.run_bass_kernel_spmd(
        st["nc"], in_maps, core_ids=list(range(N_CORES)))
    return gather(res.results, np.asarray(out_b), x.dtype)
